# revision 1
# baseline (speedup 1.0000x reference)
"""AdjacentAttention Trainium2 kernel (8-core SPMD, self-contained).

Math (restructured; projections folded on host):
  A[:, h*256+d'] = SCALE * (Wq_h Wk_h^T)[:, d']
  B[h*256+d', :] = (Wv_h Wout_h)[d', :]
  qt[i] = x[i] @ A ;  dots[i,h,j] = qt[i, h-block] . x[adj[i,j]]
  attn = softmax_j(dots)*mask/renorm ; s[i,h] = sum_j attn*x_g ; out = s@B + bout

Sharding: 1024 nodes/core, x replicated, fp16 gathers (512B rows).
Per-core: 16 banks x 64 nodes. Bank node n' = 16q + 4g + c (q,g,c in 0..3 --
wait: n' = 4*grp + c, grp = 4q+g). Edge gather order:
  pos(grp,c,j) = (grp%4)*512 + c*128 + (grp//4)*32 + j
  x_g  (plain gather):  edge at partition 32*(grp//4)+j, stripe-col (grp%4)*4+c
  x_gT (transp gather): [d%128, d//128, pos]
Dense layouts (g = grp%4 "gamma", q = grp//4):
  dots/attn  [128 rows 32c+4g+h | cols 32q+j]
  attnT      [128 rows 32q+j    | cols 32c+4g+h]   (PE transpose vs I128)
  s          [128 rows 32g+4c+h | 4q x 256]
  sT         [128 rows delta    | (q,g,c,h)]        (PE transposes vs I128)
  out        [64 rows (q,g,c)=node | 256]
qt is stored zero-PADDED per node (16 cols/node, 4 real at 4g+h) so the dots
accumulation over gamma fills dense psum rows; zeros are memset once.
"""

import numpy as np

HEADS = 4
DIM_HEAD = 64
D = 256
INNER = HEADS * DIM_HEAD
SCALE = DIM_HEAD ** -0.5
N = 8192
A_NB = 32
NCORES = 8
NSHARD = N // NCORES
NODES_PER_BANK = 64
EDGES_PER_BANK = NODES_PER_BANK * A_NB   # 2048
NGROUPS = 16


def _host_fold_weights(Wq, Wkv, Wout):
    Wq = Wq.astype(np.float64)
    Wk = Wkv[:, :INNER].astype(np.float64)
    Wv = Wkv[:, INNER:].astype(np.float64)
    Wout = Wout.astype(np.float64)
    A = np.zeros((D, HEADS * D))
    B = np.zeros((HEADS * D, D))
    for h in range(HEADS):
        sl = slice(h * DIM_HEAD, (h + 1) * DIM_HEAD)
        A[:, h * D:(h + 1) * D] = SCALE * (Wq[:, sl] @ Wk[:, sl].T)
        B[h * D:(h + 1) * D, :] = Wv[:, sl] @ Wout[sl, :]
    return A.astype(np.float32), B.astype(np.float32)


def host_prepare(x, adj, mask, Wq, Wkv, Wout, bout, nbanks_per_core=None):
    f16 = np.float16
    nbanks = nbanks_per_core or (NSHARD // NODES_PER_BANK)
    nshard = nbanks * NODES_PER_BANK
    A_m, B_m = _host_fold_weights(Wq, Wkv, Wout)
    x_f16 = np.ascontiguousarray(x.astype(f16))
    A_f16 = np.ascontiguousarray(A_m.reshape(2, 128, HEADS * D).astype(f16))
    B_f16 = np.ascontiguousarray(B_m.reshape(8, 128, D).astype(f16))
    I128 = np.eye(128, dtype=f16)

    # gather-list position of edge (grp, c, j)
    gp = np.arange(NGROUPS)[:, None, None]
    cc = np.arange(4)[None, :, None]
    jj = np.arange(A_NB)[None, None, :]
    pos = (gp % 4) * 512 + cc * 128 + (gp // 4) * 32 + jj    # [16,4,32]

    in_maps = []
    ncores = (N // nshard) if nbanks_per_core is None else NCORES
    ncores = min(ncores, NCORES)
    for core in range(ncores):
        lo = core * nshard
        xT_shard = np.ascontiguousarray(
            x[lo:lo + nshard].T.reshape(2, 128, nshard).astype(f16))

        idx16 = np.zeros((128, nbanks * EDGES_PER_BANK // 16), np.int16)
        # mask01 rows 32c+4g+h, cols b*128 + 32q + j  (dense)
        mask01 = np.zeros((128, nbanks * 128), f16)
        for b in range(nbanks):
            lst = np.zeros(EDGES_PER_BANK, np.int64)
            base = lo + b * NODES_PER_BANK
            for grp in range(NGROUPS):
                g, q = grp % 4, grp // 4
                for c in range(4):
                    node = base + grp * 4 + c
                    lst[pos[grp, c]] = adj[node]
                    for h in range(HEADS):
                        mask01[32 * c + 4 * g + h,
                               b * 128 + 32 * q: b * 128 + 32 * q + A_NB] = \
                            mask[node].astype(np.float32)
            wrapped = lst.reshape(EDGES_PER_BANK // 16, 16).T.astype(np.int16)
            col0 = b * (EDGES_PER_BANK // 16)
            idx16[:, col0:col0 + EDGES_PER_BANK // 16] = np.tile(wrapped, (8, 1))

        in_maps.append({
            "x_f16": x_f16, "xT_shard": xT_shard, "A_w": A_f16, "B_w": B_f16,
            "idx16": idx16, "mask01": mask01, "I128": I128,
            "bout": bout.astype(np.float32),
        })
    return in_maps


def make_nc(for_sim=False):
    from concourse import bacc
    if for_sim:
        return bacc.Bacc(None, target_bir_lowering=False, debug=True)
    return bacc.Bacc()


def build_core_program(nc, nbanks=None, stage=None, repeats=1):
    """stage: None=full; 'gather'|'dots'|'softmax'|'attnT'|'av'|'sT' stops after
    that phase of bank 0 and DMAs the intermediate into out (debug bisect)."""
    from contextlib import ExitStack
    import concourse.bass as bass
    import concourse.tile as tile
    from concourse import mybir
    LVL = {None: 99, "gather": 0, "dots": 1, "softmax": 2, "attnT": 3,
           "av": 4, "sT": 5}[stage]

    f16 = mybir.dt.float16
    f32 = mybir.dt.float32
    nbanks = nbanks or (NSHARD // NODES_PER_BANK)
    nshard = nbanks * NODES_PER_BANK

    x_f16 = nc.dram_tensor("x_f16", [N, D], f16, kind="ExternalInput")
    xT_shard = nc.dram_tensor("xT_shard", [2, 128, nshard], f16, kind="ExternalInput")
    A_w = nc.dram_tensor("A_w", [2, 128, HEADS * D], f16, kind="ExternalInput")
    B_w = nc.dram_tensor("B_w", [8, 128, D], f16, kind="ExternalInput")
    idx16_d = nc.dram_tensor("idx16", [128, nbanks * EDGES_PER_BANK // 16],
                             mybir.dt.int16, kind="ExternalInput")
    mask01_d = nc.dram_tensor("mask01", [128, nbanks * 128], f16, kind="ExternalInput")
    I128_d = nc.dram_tensor("I128", [128, 128], f16, kind="ExternalInput")
    bout_d = nc.dram_tensor("bout", [D], f32, kind="ExternalInput")
    out_d = nc.dram_tensor("out", [nshard, D], f32, kind="ExternalOutput")

    with tile.TileContext(nc) as tc, ExitStack() as ctx:
        singles = ctx.enter_context(tc.tile_pool(name="singles", bufs=1))
        gpool = ctx.enter_context(tc.tile_pool(name="gather", bufs=3))
        spool = ctx.enter_context(tc.tile_pool(name="soft", bufs=3))
        padpool = ctx.enter_context(tc.tile_pool(name="pads", bufs=2))
        stpool = ctx.enter_context(tc.tile_pool(name="stile", bufs=3))
        opool = ctx.enter_context(tc.tile_pool(name="outs", bufs=3))
        ps_dots = ctx.enter_context(tc.tile_pool(name="ps_dots", bufs=2, space="PSUM"))
        ps_attnT = ctx.enter_context(tc.tile_pool(name="ps_attnT", bufs=1, space="PSUM"))
        ps_s = ctx.enter_context(tc.tile_pool(name="ps_s", bufs=2, space="PSUM"))
        ps_sT = ctx.enter_context(tc.tile_pool(name="ps_sT", bufs=2, space="PSUM"))
        ps_o = ctx.enter_context(tc.tile_pool(name="ps_o", bufs=1, space="PSUM"))

        # ---------- constants ----------
        A_sb = singles.tile([128, 2, HEADS * D], f16)
        nc.sync.dma_start(out=A_sb[:, :, :], in_=A_w.rearrange("k p m -> p k m"))
        B_sb = singles.tile([128, 8, D], f16)
        nc.sync.dma_start(out=B_sb[:, :, :], in_=B_w.rearrange("k p m -> p k m"))
        xT_sb = singles.tile([128, 2, nshard], f16)
        nc.sync.dma_start(out=xT_sb[:, :, :], in_=xT_shard.rearrange("k p m -> p k m"))
        idx_sb = singles.tile([128, nbanks * EDGES_PER_BANK // 16], mybir.dt.int16)
        nc.sync.dma_start(out=idx_sb[:, :], in_=idx16_d[:, :])
        I128_sb = singles.tile([128, 128], f16)
        nc.sync.dma_start(out=I128_sb[:, :], in_=I128_d[:, :])
        bout_rep = singles.tile([64, D], f32)
        _bap = bout_d[:]
        nc.sync.dma_start(
            out=bout_rep[:, :],
            in_=bass.AP(tensor=_bap.tensor, offset=_bap.offset, ap=[[0, 64], [1, D]]))
        mask_sb = singles.tile([128, nbanks * 128], f16)
        nc.sync.dma_start(out=mask_sb[:, :], in_=mask01_d[:, :])

        # ---------- padded qt: [128 dout | dc, node*16 + slot], slot 4g+h real ----------
        qT_pad = singles.tile([128, 2, nshard * 16], f16)
        # zero whole buffer once (zeros persist; real slots overwritten below)
        third = (nshard * 16) // 4
        nc.gpsimd.memset(qT_pad[:, 0, :third * 2], 0.0)
        nc.gpsimd.memset(qT_pad[:, 0, third * 2:], 0.0)
        nc.vector.memset(qT_pad[:, 1, :third * 2], 0.0)
        nc.vector.memset(qT_pad[:, 1, third * 2:], 0.0)

        NT = min(512, nshard)
        for dc in range(2):
            for h in range(HEADS):
                for t0 in range(0, nshard, NT):
                    pt = ps_o.tile([128, NT], f32, tag="oproj")
                    for kin in range(2):
                        nc.tensor.matmul(
                            pt[:, :],
                            A_sb[:, kin, h * D + dc * 128: h * D + dc * 128 + 128],
                            xT_sb[:, kin, t0:t0 + NT],
                            start=(kin == 0), stop=(kin == 1))
                    # dst cols: n=16m+4g+c (m=n//16) -> 256m + 68g + 16c + h
                    base = qT_pad[:, dc, :]
                    dst = bass.AP(
                        tensor=base.tensor,
                        offset=base.offset + t0 * 16 + h,
                        ap=[base.ap[0], [256, NT // 16], [68, 4], [16, 4]])
                    src = pt[:, :].rearrange("p (m g c) -> p m g c", g=4, c=4)
                    nc.scalar.activation(out=dst, in_=src,
                                         func=mybir.ActivationFunctionType.Copy)

        # ---------- zero-init padded attnT variants (zeros persist) ----------
        # pads2[128 rows 32q+j | q-block(256): g-block(64): c-sub(16): 4c+h]
        # block q holds attn only in rows 32q..32q+32; all other rows stay 0,
        # so AV matmuls can be full-K (no row-group tile_position, which hangs
        # on HW when multiple row-tiles share psum partitions).
        padzero = []
        for _ in range(2):
            pz = padpool.tile([128, 1024], f16, tag="padc")
            nc.gpsimd.memset(pz[:, :], 0.0)
            padzero.append(pz)

        # pre-allocated psum tiles whose upper half-strips are never written by
        # the M=16 matmuls: zero them once (zeros persist across banks).
        dots_tiles = []
        for _ in range(2):
            dt_ = ps_dots.tile([128, 128], f32, tag="dots")
            nc.vector.memset(dt_[:, :], 0.0)
            dots_tiles.append(dt_)
        s_tiles = []
        for _ in range(2):
            st_ = ps_s.tile([128, 2, D], f32, tag="s")
            nc.vector.memset(st_[:, :, :], 0.0)
            s_tiles.append(st_)

        for b_rep in range(nbanks * repeats):
            b = b_rep % nbanks
            icol = b * (EDGES_PER_BANK // 16)
            # ---------- gathers (chunked to fit the SWDGE descriptor ring:
            # transpose <=512 idxs, plain <=1024 idxs per instruction) ----------
            xgT_chunks = []
            for g in range(4):
                xgTg = gpool.tile([128, 2, 512], f16, tag=f"xgT{g}")
                nc.gpsimd.dma_gather(
                    out_ap=xgTg[:, :, :], in_ap=x_f16[:, :],
                    idxs_ap=idx_sb[:, icol + 32 * g: icol + 32 * (g + 1)],
                    num_idxs=512, num_idxs_reg=512,
                    elem_size=D, transpose=True)
                xgT_chunks.append(xgTg)
            xg = gpool.tile([128, EDGES_PER_BANK // 128, D], f16, tag="xg")
            for m in range(2):
                nc.gpsimd.dma_gather(
                    out_ap=xg[:, 8 * m:8 * (m + 1), :], in_ap=x_f16[:, :],
                    idxs_ap=idx_sb[:, icol + 64 * m: icol + 64 * (m + 1)],
                    num_idxs=1024, num_idxs_reg=1024,
                    elem_size=D, transpose=False)

            if LVL == 0:
                dmp = opool.tile([128, 2 * D], f32, tag="obuf")
                nc.vector.tensor_copy(dmp[:, :D], xg[:, 0, :])
                nc.vector.tensor_copy(dmp[:, D:], xgT_chunks[0][:, 0, :256])
                nc.sync.dma_start(out=out_d[0:128, :].rearrange("n (a d) -> n a d", a=2) if False else out_d[0:128, :], in_=dmp[:, :D])
                break

            # ---------- dots: psum [128 (c,g,h) | 32q+j] ----------
            dots_ps = dots_tiles[b % 2]
            for q in range(4):
                for c in range(4):
                    for g in range(4):
                        grp = 4 * q + g
                        node = b * NODES_PER_BANK + grp * 4 + c
                        epos = c * 128 + q * 32
                        for dc in range(2):
                            nc.tensor.matmul(
                                dots_ps[32 * c:32 * c + 16, 32 * q:32 * q + 32],
                                qT_pad[:, dc, node * 16:(node + 1) * 16],
                                xgT_chunks[g][:, dc, epos:epos + 32],
                                start=(g == 0 and dc == 0),
                                stop=(g == 3 and dc == 1),
                                tile_position=(0, 32 * c))

            if LVL == 1:
                dmp = opool.tile([128, D], f32, tag="obuf")
                nc.vector.tensor_copy(dmp[:, :128], dots_ps[:, :])
                nc.sync.dma_start(out=out_d[0:128, :128], in_=dmp[:, :128])
                break

            # ---------- softmax (dense) ----------
            p_sb = spool.tile([128, 128], f16, tag="p")
            nc.scalar.activation(out=p_sb[:, :], in_=dots_ps[:, :],
                                 func=mybir.ActivationFunctionType.Exp)
            nc.vector.tensor_mul(p_sb[:, :], p_sb[:, :],
                                 mask_sb[:, b * 128:(b + 1) * 128])
            sm = spool.tile([128, 4], f32, tag="sm")
            nc.vector.reduce_sum(
                out=sm[:, :],
                in_=p_sb[:, :].rearrange("p (q j) -> p q j", q=4),
                axis=mybir.AxisListType.X)
            nc.vector.tensor_scalar_add(sm[:, :], sm[:, :], 1e-20)
            inv = spool.tile([128, 4], f32, tag="inv")
            nc.vector.reciprocal(out=inv[:, :], in_=sm[:, :])
            _i = inv[:, :]
            nc.vector.tensor_mul(
                p_sb[:, :].rearrange("p (q j) -> p q j", q=4),
                p_sb[:, :].rearrange("p (q j) -> p q j", q=4),
                bass.AP(tensor=_i.tensor, offset=_i.offset,
                        ap=[_i.ap[0], [1, 4], [0, 32]]))

            if LVL == 2:
                dmp = opool.tile([128, D], f32, tag="obuf")
                nc.vector.tensor_copy(dmp[:, :128], p_sb[:, :])
                nc.sync.dma_start(out=out_d[0:128, :128], in_=dmp[:, :128])
                break

            # ---------- attnT: PE transpose -> psum [128 (q,j) | (c,g,h)] ----------
            attnT_ps = ps_attnT.tile([128, 128], f16, tag="attnT")
            nc.tensor.matmul(attnT_ps[:, :], p_sb[:, :], I128_sb[:, :],
                             is_transpose=True)

            # ---------- padded attnT variants (per-q zero-isolated blocks) ----------
            pads = padzero[b % 2]
            _a = attnT_ps[:, :]
            _p = pads[:, :]
            prow = _p.ap[0][0]   # partition stride of pads
            arow = _a.ap[0][0]
            for q in range(4):
                # src rows 32q..32q+32, cols (g,c,h) = 32c+4g+h
                # dst rows 32q..32q+32, cols 256q + 64g + 20c + h
                nc.vector.tensor_copy(
                    bass.AP(tensor=_p.tensor,
                            offset=_p.offset + 32 * q * prow + 256 * q,
                            ap=[[prow, 32], [64, 4], [20, 4], [1, 4]]),
                    bass.AP(tensor=_a.tensor,
                            offset=_a.offset + 32 * q * arow,
                            ap=[[arow, 32], [4, 4], [32, 4], [1, 4]]))

            if LVL == 3:
                dmp = opool.tile([128, D], f32, tag="obuf")
                nc.vector.tensor_copy(dmp[:, :64], pads[:, 64:128])
                nc.vector.tensor_copy(dmp[:, 64:192], attnT_ps[:, :])
                nc.sync.dma_start(out=out_d[0:128, :192], in_=dmp[:, :192])
                break

            # ---------- AV: s psum 2 half-tiles [128 (g,c,h) | 2q x 256] ----------
            s_sb = stpool.tile([128, 4, D], f16, tag="snat")
            for qh in range(2):
                s_ps = s_tiles[qh]
                for q2 in range(2):
                    q = 2 * qh + q2
                    for g in range(4):
                        for c in range(4):
                            off = 256 * q + 64 * g + 16 * c
                            nc.tensor.matmul(
                                s_ps[32 * g:32 * g + 16, q2, :],
                                pads[:, off:off + 16],
                                xg[:, 4 * g + c, :],
                                start=(c == 0), stop=(c == 3),
                                tile_position=(0, 32 * g))
                nc.scalar.activation(
                    out=s_sb[:, 2 * qh:2 * qh + 2, :], in_=s_ps[:, :, :],
                    func=mybir.ActivationFunctionType.Copy)

            if LVL == 4:
                dmp = opool.tile([128, D], f32, tag="obuf")
                nc.vector.tensor_copy(dmp[:, :], s_sb[:, 0, :])
                nc.sync.dma_start(out=out_d[0:128, :], in_=dmp[:, :])
                break
            # sT2 cols: dh*256 + h*64 + (16q+4g+c)  -> head slices contiguous
            sT_sb = stpool.tile([128, 2, 4, 64], f16, tag="sT")
            for q in range(4):
                for dh in range(2):
                    tps = ps_sT.tile([128, 128], f16, tag="sTp")
                    nc.tensor.matmul(
                        tps[:, :], s_sb[:, q, dh * 128:(dh + 1) * 128],
                        I128_sb[:, :], is_transpose=True)
                    # src cols (g,c,h) = 32g+4c+h ; dst col h*64 + 16q+4g+c
                    _t = sT_sb[:, :, :, :]
                    dst = bass.AP(
                        tensor=_t.tensor, offset=_t.offset + dh * 256 + 16 * q,
                        ap=[_t.ap[0], [4, 4], [1, 4], [64, 4]])
                    _s = tps[:, :]
                    src = bass.AP(tensor=_s.tensor, offset=_s.offset,
                                  ap=[_s.ap[0], [32, 4], [4, 4], [1, 4]])
                    nc.vector.tensor_copy(dst, src)

            # ---------- out projection ----------
            # sT cols: 128q + 32g + 4c + h ; node row m = (q,g,c) = 16q+4g+c
            o_ps = ps_o.tile([64, D], f32, tag="oproj")
            for h in range(HEADS):
                for dh in range(2):
                    kidx = 2 * h + dh
                    nc.tensor.matmul(o_ps[:, :],
                                     sT_sb[:, dh, h, :],
                                     B_sb[:, kidx, :],
                                     start=(kidx == 0), stop=(kidx == 7))
            o_sb = opool.tile([64, D], f32, tag="obuf")
            nc.vector.tensor_add(o_sb[:, :], o_ps[:, :], bout_rep[:, :])
            nc.sync.dma_start(
                out=out_d[b * NODES_PER_BANK:(b + 1) * NODES_PER_BANK, :],
                in_=o_sb[:, :])
    return nc


def kernel(**inputs):
    import sys
    if "/opt/trn_rl_repo" not in sys.path:
        sys.path.insert(0, "/opt/trn_rl_repo")
    from concourse.bass_utils import run_bass_kernel_spmd

    x = np.asarray(inputs["x"])[0].astype(np.float32)
    adj = np.asarray(inputs["adj_kv_indices"])[0]
    mask = np.asarray(inputs["mask"])[0]
    in_maps = host_prepare(x, adj, mask,
                           np.asarray(inputs["Wq"]), np.asarray(inputs["Wkv"]),
                           np.asarray(inputs["Wout"]), np.asarray(inputs["bout"]))
    nc = make_nc()
    build_core_program(nc)
    if not nc.is_finalized():
        nc.finalize()
    res = run_bass_kernel_spmd(nc, in_maps, list(range(NCORES)))
    out = np.concatenate([np.asarray(res.results[i]["out"]) for i in range(NCORES)],
                         axis=0)
    return out[None].astype(np.float32)



# revision 11
# speedup vs baseline: 1.9861x; 1.9861x over previous
"""AdjacentAttention Trainium2 kernel (8-core SPMD, self-contained).

Math (projections folded on host):
  A[:, h*256+d'] = SCALE * (Wq_h Wk_h^T)[:, d']      (d x 1024)
  B[h*256+d', :] = (Wv_h Wout_h)[d', :]              (1024 x d)
  qt[n] = x[n] @ A ;  dots[n,h,j] = qt[n, h-block] . x[adj[n,j]]
  attn = softmax_j(dots) * mask / renorm
  s[n,h] = sum_j attn * x_gathered ;  out = s @ B + bout

Sharding: 1024 nodes/core, x replicated in HBM (fp16), 16 banks x 64 nodes.
Bank-local node n' = 16q + cid, cid = 4g + c (q,g,c in 0..3).  Gather order:
  pos(n', j) = 128*cid + 32*q + j
  xgT (transposed gather): [d%128, d//128, pos]   (d on partitions)
  xg  (plain gather):      [pos%128, pos//128, d] (edges on partitions)

Everything is structured so the PE moving operand is small (cost model:
matmul time = moving cols x PE cycle, M/K free):
  dotsT[e=(q,j), slot=(q',h)] per chunk cid: stationary xgT chunk [128d x 128e],
    moving qT slice [128d x 16 slots] -> psum [128 e, 16], accum over dc.
  softmax in this layout: exp (Act) -> * maskT (DVE; maskT also zeroes the
    off-diagonal q!=q' slots) -> q-block sums via block-ones matmul (PE)
    -> reciprocal (DVE) -> renorm mul (DVE).
  AV: stationary xg chunk [128e x 128d-half], moving attn chunk [128e x 16]
    -> psum sT [128 d-half, slot].
  sT -> SBUF with (q,cid,h)->(64q+4cid+h) reorder so out-proj stationary is
    a stride-4 AP over 128 nodes (2 banks): out = sT^T @ B accum over (h,dc),
    + K=1 ones x bout matmul for the bias.
"""

import numpy as np

HEADS = 4
DIM_HEAD = 64
D = 256
INNER = HEADS * DIM_HEAD
SCALE = DIM_HEAD ** -0.5
N = 8192
A_NB = 32
NCORES = 8
NSHARD = N // NCORES
NODES_PER_BANK = 64
EDGES_PER_BANK = NODES_PER_BANK * A_NB   # 2048
NCHUNK = 16                              # 128-edge chunks per bank


def _host_fold_weights(Wq, Wkv, Wout):
    Wq = Wq.astype(np.float64)
    Wk = Wkv[:, :INNER].astype(np.float64)
    Wv = Wkv[:, INNER:].astype(np.float64)
    Wout = Wout.astype(np.float64)
    A = np.zeros((D, HEADS * D))
    B = np.zeros((HEADS * D, D))
    for h in range(HEADS):
        sl = slice(h * DIM_HEAD, (h + 1) * DIM_HEAD)
        A[:, h * D:(h + 1) * D] = SCALE * (Wq[:, sl] @ Wk[:, sl].T)
        B[h * D:(h + 1) * D, :] = Wv[:, sl] @ Wout[sl, :]
    return A.astype(np.float32), B.astype(np.float32)


def host_prepare(x, adj, mask, Wq, Wkv, Wout, bout, nbanks_per_core=None):
    f16 = np.float16
    nbanks = nbanks_per_core or (NSHARD // NODES_PER_BANK)
    nshard = nbanks * NODES_PER_BANK
    A_m, B_m = _host_fold_weights(Wq, Wkv, Wout)
    x_f16 = np.ascontiguousarray(x.astype(f16))
    A_f16 = np.ascontiguousarray(A_m.reshape(2, 128, HEADS * D).astype(f16))
    B_f16 = np.ascontiguousarray(B_m.reshape(8, 128, D).astype(f16))

    # all-ones: full-column sums of the masked exp ARE the per-(node,head)
    # denominators (mask zeroes the off-diagonal q blocks), replicated to
    # every output partition -- exactly the broadcast the renorm mul needs.
    qones = np.ones((128, 128), f16)
    ones1 = np.ones((1, 128), f16)
    bout16 = bout.astype(f16).reshape(1, D)

    # gather-list position of edge (n'=16q+cid, j): 128*cid + 32*q + j
    qq = np.arange(4)[:, None, None]
    ci = np.arange(4 * 4)[None, :, None]
    jj = np.arange(A_NB)[None, None, :]
    pos = 128 * ci + 32 * qq + jj      # [q, cid, j]

    in_maps = []
    ncores = (N // nshard) if nbanks_per_core is None else NCORES
    ncores = min(ncores, NCORES)
    for core in range(ncores):
        lo = core * nshard
        xT_shard = np.ascontiguousarray(
            x[lo:lo + nshard].T.reshape(2, 128, nshard).astype(f16))

        idx16 = np.zeros((128, nbanks * EDGES_PER_BANK // 16), np.int16)
        # maskT: [row 32q+j | col b*256 + 16*cid + 4q + h]
        maskT = np.zeros((128, nbanks * 256), f16)
        for b in range(nbanks):
            lst = np.zeros(EDGES_PER_BANK, np.int64)
            base = lo + b * NODES_PER_BANK
            for q in range(4):
                for cid in range(NCHUNK):
                    node = base + 16 * q + cid
                    lst[pos[q, cid]] = adj[node]
                    for h in range(HEADS):
                        maskT[32 * q:32 * q + A_NB,
                              b * 256 + 16 * cid + 4 * q + h] = \
                            mask[node].astype(np.float32)
            wrapped = lst.reshape(EDGES_PER_BANK // 16, 16).T.astype(np.int16)
            col0 = b * (EDGES_PER_BANK // 16)
            idx16[:, col0:col0 + EDGES_PER_BANK // 16] = np.tile(wrapped, (8, 1))

        in_maps.append({
            "x_f16": x_f16, "xT_shard": xT_shard, "A_w": A_f16, "B_w": B_f16,
            "idx16": idx16, "maskT": maskT, "qones": qones, "ones1": ones1,
            "bout16": bout16, "I128": np.eye(128, dtype=f16),
        })
    return in_maps


def make_nc(for_sim=False):
    from concourse import bacc
    kw = dict(dynamic_dma_scratch_size=96 * 1024)
    if for_sim:
        return bacc.Bacc(None, target_bir_lowering=False, debug=True, **kw)
    return bacc.Bacc(**kw)


def build_core_program(nc, nbanks=None, stage=None, repeats=1):
    """stage: None=full; 'gather'|'dots'|'softmax'|'av' stops after that phase
    of bank 0 and DMAs the intermediate into out (debug bisect)."""
    from contextlib import ExitStack
    import concourse.bass as bass
    import concourse.tile as tile
    from concourse import mybir
    LVL = {None: 99, "gather": 0, "dots": 1, "softmax": 2, "av": 3}[stage]

    f16 = mybir.dt.float16
    f32 = mybir.dt.float32
    nbanks = nbanks or (NSHARD // NODES_PER_BANK)
    nshard = nbanks * NODES_PER_BANK

    x_f16 = nc.dram_tensor("x_f16", [N, D], f16, kind="ExternalInput")
    xT_shard = nc.dram_tensor("xT_shard", [2, 128, nshard], f16, kind="ExternalInput")
    A_w = nc.dram_tensor("A_w", [2, 128, HEADS * D], f16, kind="ExternalInput")
    B_w = nc.dram_tensor("B_w", [8, 128, D], f16, kind="ExternalInput")
    idx16_d = nc.dram_tensor("idx16", [128, nbanks * EDGES_PER_BANK // 16],
                             mybir.dt.int16, kind="ExternalInput")
    maskT_d = nc.dram_tensor("maskT", [128, nbanks * 256], f16, kind="ExternalInput")
    qones_d = nc.dram_tensor("qones", [128, 128], f16, kind="ExternalInput")
    ones1_d = nc.dram_tensor("ones1", [1, 128], f16, kind="ExternalInput")
    bout16_d = nc.dram_tensor("bout16", [1, D], f16, kind="ExternalInput")
    I128_d = nc.dram_tensor("I128", [128, 128], f16, kind="ExternalInput")
    out_d = nc.dram_tensor("out", [nshard, D], f32, kind="ExternalOutput")

    with tile.TileContext(nc) as tc, ExitStack() as ctx:
        singles = ctx.enter_context(tc.tile_pool(name="singles", bufs=1))
        gpool = ctx.enter_context(tc.tile_pool(name="gather", bufs=3))
        spool = ctx.enter_context(tc.tile_pool(name="soft", bufs=3))
        stpool = ctx.enter_context(tc.tile_pool(name="stile", bufs=2))
        opool = ctx.enter_context(tc.tile_pool(name="outs", bufs=2))
        xtpool = ctx.enter_context(tc.tile_pool(name="xgt", bufs=2))
        ps_qt = ctx.enter_context(tc.tile_pool(name="ps_qt", bufs=1, space="PSUM"))
        ps_ds = ctx.enter_context(tc.tile_pool(name="ps_ds", bufs=2, space="PSUM"))
        ps_sT = ctx.enter_context(tc.tile_pool(name="ps_sT", bufs=2, space="PSUM"))
        ps_o = ctx.enter_context(tc.tile_pool(name="ps_o", bufs=1, space="PSUM"))
        ps_tr = ctx.enter_context(tc.tile_pool(name="ps_tr", bufs=2, space="PSUM"))

        # ---------- constants ----------
        A_sb = singles.tile([128, 2, HEADS * D], f16)
        nc.sync.dma_start(out=A_sb[:, :, :], in_=A_w.rearrange("k p m -> p k m"))
        B_sb = singles.tile([128, 8, D], f16)
        nc.sync.dma_start(out=B_sb[:, :, :], in_=B_w.rearrange("k p m -> p k m"))
        xT_sb = singles.tile([128, 2, nshard], f16)
        nc.sync.dma_start(out=xT_sb[:, :, :], in_=xT_shard.rearrange("k p m -> p k m"))
        idx_sb = singles.tile([128, nbanks * EDGES_PER_BANK // 16], mybir.dt.int16)
        nc.sync.dma_start(out=idx_sb[:, :], in_=idx16_d[:, :])
        maskT_sb = singles.tile([128, nbanks * 256], f16)
        nc.sync.dma_start(out=maskT_sb[:, :], in_=maskT_d[:, :])
        qones_sb = singles.tile([128, 128], f16)
        nc.sync.dma_start(out=qones_sb[:, :], in_=qones_d[:, :])
        ones1_sb = singles.tile([1, 128], f16)
        nc.sync.dma_start(out=ones1_sb[:, :], in_=ones1_d[:, :])
        bout_sb = singles.tile([1, D], f16)
        nc.sync.dma_start(out=bout_sb[:, :], in_=bout16_d[:, :])
        I128_sb = singles.tile([128, 128], f16)
        nc.sync.dma_start(out=I128_sb[:, :], in_=I128_d[:, :])

        # ---------- qT: [128 d'-half | dc, 4*node + h] ----------
        qT = singles.tile([128, 2, nshard * HEADS], f16)
        NT = min(512, nshard)
        for dc in range(2):
            for h in range(HEADS):
                for t0 in range(0, nshard, NT):
                    pt = ps_qt.tile([128, NT], f32, tag="qt")
                    for kin in range(2):
                        nc.tensor.matmul(
                            pt[:, :],
                            A_sb[:, kin, h * D + dc * 128: h * D + dc * 128 + 128],
                            xT_sb[:, kin, t0:t0 + NT],
                            start=(kin == 0), stop=(kin == 1))
                    base = qT[:, dc, :]
                    dst = bass.AP(
                        tensor=base.tensor,
                        offset=base.offset + 4 * t0 + h,
                        ap=[base.ap[0], [4, NT]])
                    nc.scalar.activation(out=dst, in_=pt[:, :],
                                         func=mybir.ActivationFunctionType.Copy)

        for b_rep in range(nbanks * repeats):
            b = b_rep % nbanks
            icol = b * (EDGES_PER_BANK // 16)
            # ---------- gather (plain only; HW caps 1024 idxs/instruction) ----------
            xg = gpool.tile([128, NCHUNK, D], f16, tag="xg")
            for m in range(2):
                nc.gpsimd.dma_gather(
                    out_ap=xg[:, 8 * m:8 * (m + 1), :], in_ap=x_f16[:, :],
                    idxs_ap=idx_sb[:, icol + 64 * m: icol + 64 * (m + 1)],
                    num_idxs=1024, num_idxs_reg=1024,
                    elem_size=D, transpose=False)

            # ---------- xgT = transpose(xg) on PE, copies split DVE/Act ----------
            xgT = xtpool.tile([128, 2, EDGES_PER_BANK], f16, tag="xgT")
            for dc in range(2):
                for grp4 in range(4):
                    tr_ps = ps_tr.tile([128, 512], f16, tag="tr")
                    for i in range(4):
                        nc.tensor.matmul(
                            tr_ps[:, 128 * i:128 * (i + 1)],
                            xg[:, 4 * grp4 + i, 128 * dc:128 * (dc + 1)],
                            I128_sb[:, :], is_transpose=True)
                    dstc = xgT[:, dc, 512 * grp4:512 * (grp4 + 1)]
                    if grp4 % 2 == 0:
                        nc.vector.tensor_copy(dstc, tr_ps[:, :])
                    else:
                        nc.scalar.activation(
                            out=dstc, in_=tr_ps[:, :],
                            func=mybir.ActivationFunctionType.Copy)

            if LVL == 0:
                dmp = opool.tile([128, 2 * D], f32, tag="obuf")
                nc.vector.tensor_copy(dmp[:, :D], xg[:, 0, :])
                nc.vector.tensor_copy(dmp[:, D:], xgT[:, 0, :256])
                nc.sync.dma_start(out=out_d[0:128, :], in_=dmp[:, :D])
                break

            # ---------- dotsT: psum [128 (32q+j) | 16*cid + 4q + h] ----------
            ds_ps = ps_ds.tile([128, 512], f32, tag="ds")
            dt_ps = ds_ps[:, 0:256]
            for cid in range(NCHUNK):
                qbase = qT[:, 0, :]
                for dc in range(2):
                    qsl = qT[:, dc, :]
                    nc.tensor.matmul(
                        dt_ps[:, 16 * cid:16 * cid + 16],
                        xgT[:, dc, 128 * cid:128 * (cid + 1)],
                        bass.AP(tensor=qsl.tensor,
                                offset=qsl.offset + 256 * b + 4 * cid,
                                ap=[qsl.ap[0], [64, 4], [1, 4]]),
                        start=(dc == 0), stop=(dc == 1))

            if LVL == 1:
                dmp = opool.tile([128, D], f32, tag="obuf")
                nc.vector.tensor_copy(dmp[:, :], dt_ps[:, :])
                nc.sync.dma_start(out=out_d[0:128, :], in_=dmp[:, :])
                break

            # ---------- softmax (transposed layout) ----------
            p_sb = spool.tile([128, 256], f16, tag="p")
            nc.scalar.activation(out=p_sb[:, :], in_=dt_ps[:, :],
                                 func=mybir.ActivationFunctionType.Exp)
            nc.vector.tensor_mul(p_sb[:, :], p_sb[:, :],
                                 maskT_sb[:, b * 256:(b + 1) * 256])
            sums_ps = ds_ps[:, 256:512]
            nc.tensor.matmul(sums_ps[:, :], qones_sb[:, :], p_sb[:, :],
                             start=True, stop=True)
            inv_sb = spool.tile([128, 256], f32, tag="inv")
            nc.vector.reciprocal(out=inv_sb[:, :], in_=sums_ps[:, :])
            nc.vector.tensor_mul(p_sb[:, :], p_sb[:, :], inv_sb[:, :])

            if LVL == 2:
                dmp = opool.tile([128, D], f32, tag="obuf")
                nc.vector.tensor_copy(dmp[:, :], p_sb[:, :])
                nc.sync.dma_start(out=out_d[0:128, :], in_=dmp[:, :])
                break

            # ---------- AV: psum sT [128 d-half | dc, 16*cid + 4q + h] ----------
            sT_ps = ps_sT.tile([128, 2, 256], f32, tag="sT")
            for dc in range(2):
                for cid in range(NCHUNK):
                    nc.tensor.matmul(
                        sT_ps[:, dc, 16 * cid:16 * cid + 16],
                        xg[:, cid, 128 * dc:128 * (dc + 1)],
                        p_sb[:, 16 * cid:16 * cid + 16],
                        start=True, stop=True)

            # sT -> SBUF, reorder (q,cid,h) -> 64q + 4cid + h; 2 banks/block
            bb = b % 2
            if bb == 0:
                sT_sb = stpool.tile([128, 2, 2 * 256], f16, tag="sTsb")
            for dc in range(2):
                src0 = sT_ps[:, dc, :]
                src = bass.AP(tensor=src0.tensor, offset=src0.offset,
                              ap=[src0.ap[0], [4, 4], [16, NCHUNK], [1, 4]])
                dst0 = sT_sb[:, dc, :]
                dst = bass.AP(tensor=dst0.tensor, offset=dst0.offset + 256 * bb,
                              ap=[dst0.ap[0], [64, 4], [4, NCHUNK], [1, 4]])
                nc.scalar.activation(out=dst, in_=src,
                                     func=mybir.ActivationFunctionType.Copy)

            if LVL == 3:
                dmp = opool.tile([128, D], f32, tag="obuf")
                nc.vector.tensor_copy(dmp[:, :], sT_ps[:, 0, :])
                nc.sync.dma_start(out=out_d[0:128, :], in_=dmp[:, :])
                break

            # ---------- out projection per 2-bank block ----------
            if bb == 1:
                o_ps = ps_o.tile([128, D], f32, tag="oproj")
                for kidx in range(8):
                    h, dc = kidx // 2, kidx % 2
                    st0 = sT_sb[:, dc, :]
                    nc.tensor.matmul(
                        o_ps[:, :],
                        bass.AP(tensor=st0.tensor, offset=st0.offset + h,
                                ap=[st0.ap[0], [4, 128]]),
                        B_sb[:, kidx, :],
                        start=(kidx == 0), stop=False)
                nc.tensor.matmul(o_ps[:, :], ones1_sb[:, :], bout_sb[:, :],
                                 start=False, stop=True)
                o_sb = opool.tile([128, D], f32, tag="obuf")
                nc.scalar.activation(out=o_sb[:, :], in_=o_ps[:, :],
                                     func=mybir.ActivationFunctionType.Copy)
                nc.sync.dma_start(
                    out=out_d[(b - 1) * NODES_PER_BANK:(b + 1) * NODES_PER_BANK, :],
                    in_=o_sb[:, :])
    return nc


def kernel(**inputs):
    import sys
    if "/opt/trn_rl_repo" not in sys.path:
        sys.path.insert(0, "/opt/trn_rl_repo")
    from concourse.bass_utils import run_bass_kernel_spmd

    x = np.asarray(inputs["x"])[0].astype(np.float32)
    adj = np.asarray(inputs["adj_kv_indices"])[0]
    mask = np.asarray(inputs["mask"])[0]
    in_maps = host_prepare(x, adj, mask,
                           np.asarray(inputs["Wq"]), np.asarray(inputs["Wkv"]),
                           np.asarray(inputs["Wout"]), np.asarray(inputs["bout"]))
    nc = make_nc()
    build_core_program(nc)
    if not nc.is_finalized():
        nc.finalize()
    res = run_bass_kernel_spmd(nc, in_maps, list(range(NCORES)))
    out = np.concatenate([np.asarray(res.results[i]["out"]) for i in range(NCORES)],
                         axis=0)
    return out[None].astype(np.float32)
